# revision 1
# baseline (speedup 1.0000x reference)
"""2D Haar DWT (analysis) kernel for Trainium2, 8 NeuronCores.

Reference computation: per (batch, channel) slice, Y = A @ X @ A.T with A the
512x512 single-level Haar analysis operator (2-tap filters h0=[s,s],
h1=[-s,s], s=1/sqrt(2), stride 2, no wrap for L=2), then the four quadrants
of Y are concatenated along channels: out[b,i,j,:] = [LL|LH|HL|HH].

Because A is 2-tap / stride-2, every output pixel is a +-s^2-weighted sum of
one 2x2 input block:
    hs = x[2i]   + x[2i+1]        (height lowpass,  unscaled)
    hd = x[2i+1] - x[2i]          (height highpass, unscaled)
    LL = k*(hs[2j] + hs[2j+1])    k = s*s
    LH = k*(hd[2j] + hd[2j+1])
    HL = k*(hs[2j+1] - hs[2j])
    HH = k*(hd[2j+1] - hd[2j])
so the kernel is pure elementwise work (memory-bound), no matmul needed.
The host verifies that A has exactly this structure (it is deterministic in
the problem's setup_inputs); if it ever did not, a numpy fallback computes
the general dense transform.

Sharding: data-parallel over batch, 1 image per NeuronCore (8 cores).

Raw bass (no Tile): this container's walrus accepts at most one sync-wait
command per instruction, which the Tile scheduler's emitted sync_info
violates; here every instruction carries at most one sem wait by
construction. Pipeline: SP issues loads (HWDGE), DVE does the 6 adds/subs
per tile, ACT applies the k scale and issues stores (HWDGE), GPSIMD waits
for everything and resets all semaphores so repeated NEFF executions start
from a clean state.
"""

from contextlib import ExitStack

import numpy as np

import concourse.bass as bass
import concourse.mybir as mybir
from concourse import bass_utils
from concourse.instruction_name_ordered_set import InstructionNameOrderedSet


class _Chain:
    """Declare same-engine program-order as nosync dependencies (what Tile
    emits) so the race detector knows consecutive ops on one engine are
    ordered by the engine itself."""

    def __init__(self):
        self.prev = None

    def __call__(self, inst):
        if self.prev is not None:
            inst.ins.set_nosync_dependencies(
                InstructionNameOrderedSet([self.prev])
            )
        self.prev = inst.ins.name
        return inst

_B = 8
_N = 512
_C = 32
_HALF = _N // 2

# tile geometry (per core)
_IB = 2          # i-blocks of 128 output rows each (256 total)
_WCH = 64        # input-width columns per tile
_WB = _N // _WCH # w-chunks
_JCH = _WCH // 2 # output columns per tile

_NB_EO = 4       # input-tile buffers (load lookahead)
_NB_OT = 3       # output-tile buffers

_f32 = mybir.dt.float32
_ADD = mybir.AluOpType.add
_SUB = mybir.AluOpType.subtract


def _build_nc(k: float, repeat: int = 1) -> bass.Bass:
    """Build the per-core kernel. repeat>1 runs the whole DWT that many
    times inside one NEFF (identical output) — used only for timing via
    the wall-clock slope between repeat values."""
    nc = bass.Bass()
    x = nc.dram_tensor("x", [_N, _N, _C], _f32, kind="ExternalInput")
    out = nc.dram_tensor("out", [_HALF, _HALF, 4 * _C], _f32, kind="ExternalOutput")

    # view x rows as (i, even/odd) pairs
    xr = x[:].rearrange("(i e) w c -> i e w c", e=2)  # [256, 2, 512, 32]

    units = [
        (ib, wb) for _ in range(repeat) for ib in range(_IB) for wb in range(_WB)
    ]
    n_units = len(units)

    with ExitStack() as ctx:
        eo = [
            ctx.enter_context(nc.sbuf_tensor(f"eo{i}", [128, 2, _WCH, _C], _f32))
            for i in range(_NB_EO)
        ]
        hs = ctx.enter_context(nc.sbuf_tensor("hs", [128, _WCH, _C], _f32))
        hd = ctx.enter_context(nc.sbuf_tensor("hd", [128, _WCH, _C], _f32))
        ot = [
            ctx.enter_context(nc.sbuf_tensor(f"ot{i}", [128, _JCH, 4, _C], _f32))
            for i in range(_NB_OT)
        ]
        # One load sem per eo slot and one store sem per ot slot: DMA
        # completions across queues are not ordered, so a single cumulative
        # counter could wake a waiter when a *different* load finished. With
        # per-slot lanes (and the slot-free backpressure keeping at most one
        # in-flight DMA per lane) each wait identifies exactly its transfer.
        s_load = [
            ctx.enter_context(nc.semaphore(f"s_load{i}")) for i in range(_NB_EO)
        ]
        s_store = [
            ctx.enter_context(nc.semaphore(f"s_store{i}")) for i in range(_NB_OT)
        ]
        s_eofree = ctx.enter_context(nc.semaphore("s_eofree"))   # DVE done reading eo buf
        s_otready = ctx.enter_context(nc.semaphore("s_otready")) # DVE done writing ot buf
        s_mul = ctx.enter_context(nc.semaphore("s_mul"))         # ACT mul datapath done
        s_bar = ctx.enter_context(nc.semaphore("s_bar"))         # end-of-stream barrier
        block = ctx.enter_context(nc.Block())

        sems = s_load + s_store + [s_eofree, s_otready, s_mul, s_bar]
        n_store_lane = [len(range(lane, n_units, _NB_OT)) for lane in range(_NB_OT)]

        @block.sync
        def _(sync):
            ch = _Chain()
            for u, (ib, wb) in enumerate(units):
                src = xr[
                    ib * 128 : (ib + 1) * 128, :, wb * _WCH : (wb + 1) * _WCH, :
                ]
                i = ch(sync.dma_start(out=eo[u % _NB_EO][:], in_=src))
                if u >= _NB_EO:
                    i.wait_op(s_eofree, u - _NB_EO + 1, "sem-ge")
                i.then_inc(s_load[u % _NB_EO], 16)
            ch(sync.sem_inc(s_bar, 1))

        @block.vector
        def _(vector):
            ch = _Chain()
            for u, (ib, wb) in enumerate(units):
                b = eo[u % _NB_EO]
                o = ot[u % _NB_OT]
                ev = b[:, 0]  # [128, WCH, C]
                od = b[:, 1]
                ch(vector.tensor_tensor(out=hs[:], in0=ev, in1=od, op=_ADD)).wait_op(
                    s_load[u % _NB_EO], 16 * (u // _NB_EO + 1), "sem-ge"
                )
                ch(vector.tensor_tensor(out=hd[:], in0=od, in1=ev, op=_SUB)).then_inc(
                    s_eofree, 1
                )
                sv = hs[:].rearrange("p (j e) c -> p j e c", e=2)
                dv = hd[:].rearrange("p (j e) c -> p j e c", e=2)
                quads = (
                    (sv[:, :, 0], sv[:, :, 1], _ADD),  # LL
                    (dv[:, :, 0], dv[:, :, 1], _ADD),  # LH
                    (sv[:, :, 1], sv[:, :, 0], _SUB),  # HL
                    (dv[:, :, 1], dv[:, :, 0], _SUB),  # HH
                )
                for qi, (a, bb, op) in enumerate(quads):
                    i = ch(vector.tensor_tensor(out=o[:, :, qi], in0=a, in1=bb, op=op))
                    if qi == 0 and u >= _NB_OT:
                        # ot slot reuse: wait until its previous store landed
                        i.wait_op(
                            s_store[u % _NB_OT], 16 * (u // _NB_OT), "sem-ge"
                        )
                i.then_inc(s_otready, 1)
            ch(vector.sem_inc(s_bar, 1))

        @block.scalar
        def _(scalar):
            ch = _Chain()
            for u, (ib, wb) in enumerate(units):
                o = ot[u % _NB_OT]
                otf = o[:].rearrange("p j q c -> p (j q c)")
                i = ch(scalar.mul(otf, otf, k)).wait_op(s_otready, u + 1, "sem-ge")
                i.then_inc(s_mul, 1)
                dst = out[
                    ib * 128 : (ib + 1) * 128, wb * _JCH : (wb + 1) * _JCH, :
                ]
                # the HWDGE trigger would otherwise race the ACT datapath
                ch(scalar.dma_start(
                    out=dst, in_=o[:].rearrange("p j q c -> p j (q c)")
                )).wait_op(s_mul, u + 1, "sem-ge").then_inc(s_store[u % _NB_OT], 16)
            ch(scalar.sem_inc(s_bar, 1))

        @block.gpsimd
        def _(gpsimd):
            ch = _Chain()
            ch(gpsimd.wait_ge(s_bar, 3))
            for lane in range(_NB_OT):
                ch(gpsimd.wait_ge(s_store[lane], 16 * n_store_lane[lane]))
            # observe every semaphore's final value before resetting them
            for lane in range(_NB_EO):
                ch(gpsimd.wait_ge(s_load[lane], 16 * len(range(lane, n_units, _NB_EO))))
            ch(gpsimd.wait_ge(s_eofree, n_units))
            ch(gpsimd.wait_ge(s_otready, n_units))
            ch(gpsimd.wait_ge(s_mul, n_units))
            nums = sorted(s.num for s in sems)
            lo = nums[0]
            hi = nums[-1] + 1
            assert nums == list(range(lo, hi)), nums
            ch(gpsimd.dma_reset(range(lo, hi)))
            ch(gpsimd.sem_clear(range(lo, hi)))

    return nc


def _expected_A(s: np.float32) -> np.ndarray:
    A = np.zeros((_N, _N), np.float32)
    i = np.arange(_HALF)
    A[i, 2 * i] = s
    A[i, 2 * i + 1] = s
    A[_HALF + i, 2 * i] = -s
    A[_HALF + i, 2 * i + 1] = s
    return A


def _fallback(x: np.ndarray, A: np.ndarray) -> np.ndarray:
    # dense separable transform, mirrors the reference in fp32
    xt = np.transpose(x, (0, 2, 1, 3))
    y = np.einsum("ij,bjkc->bikc", A, xt, optimize=True).astype(np.float32)
    y = np.transpose(y, (0, 2, 1, 3))
    y = np.einsum("ij,bjkc->bikc", A, y, optimize=True).astype(np.float32)
    mid = y.shape[1] // 2
    return np.concatenate(
        [y[:, :mid, :mid], y[:, mid:, :mid], y[:, :mid, mid:], y[:, mid:, mid:]],
        axis=-1,
    )


def run_on_device(x: np.ndarray, k: float, trace: bool = False):
    """Run the Bass kernel on 8 cores. Returns (out [8,256,256,128], results)."""
    nc = _build_nc(k)
    in_maps = [{"x": np.ascontiguousarray(x[b])} for b in range(_B)]
    res = bass_utils.run_bass_kernel_spmd(
        nc, in_maps, core_ids=list(range(_B)), trace=trace
    )
    out = np.stack([r["out"] for r in res.results], axis=0)
    return out, res


def kernel(x: np.ndarray, A: np.ndarray) -> np.ndarray:
    x = np.asarray(x, dtype=np.float32)
    A = np.asarray(A, dtype=np.float32)
    s = A[0, 0]
    if not np.array_equal(A, _expected_A(s)):
        return _fallback(x, A)
    k = float(np.float32(s) * np.float32(s))
    out, _ = run_on_device(x, k)
    return out



# revision 2
# speedup vs baseline: 1.0306x; 1.0306x over previous
"""2D Haar DWT (analysis) kernel for Trainium2, 8 NeuronCores — V3.

Same math as the baseline (per (batch, channel) slice Y = A @ X @ A.T with
the 2-tap/stride-2 Haar operator, quadrants concatenated on channels), same
sharding (1 image per core), but restructured for engine balance + overlap:

  stage 1 (vertical):   hs = x[2i] + x[2i+1]   (DVE)
                        hd = x[2i+1] - x[2i]   (Pool/GpSimd — offload)
        both written into one tile H[128, w, 2(s|d), c] so that
  stage 2 (horizontal): LLLH = H[2j] + H[2j+1] (DVE) -> O[:, j, 0:2, c]
                        HLHH = H[2j+1] - H[2j] (DVE) -> O[:, j, 2:4, c]
        two ops produce all four quadrants in the output channel order
        (q-major = horizontal sign, s|d minor = vertical sign).
  scale (ACT):          O *= s*s, then ACT triggers the store.

Engine budget per core (model): DMA 158us (in+out at ~423 GB/s), DVE
~110us, Pool ~72us, ACT ~70us -> DMA-bound when pipelined.

Raw bass; every instruction carries at most one sem wait (walrus limit),
extra conditions use standalone engine wait_ge instructions.
"""

from contextlib import ExitStack

import numpy as np

import concourse.bass as bass
import concourse.mybir as mybir
from concourse import bass_utils
from concourse.instruction_name_ordered_set import InstructionNameOrderedSet


class _Chain:
    """Declare same-engine program-order as nosync dependencies so the race
    detector knows consecutive ops on one engine are ordered."""

    def __init__(self):
        self.prev = None

    def __call__(self, inst):
        if self.prev is not None:
            inst.ins.set_nosync_dependencies(
                InstructionNameOrderedSet([self.prev])
            )
        self.prev = inst.ins.name
        return inst


_B = 8
_N = 512
_C = 32
_HALF = _N // 2

# tile geometry (per core)
_IB = 2          # i-blocks of 128 output rows each (256 total)
_WCH = 64        # input-width columns per tile
_WB = _N // _WCH # w-chunks per i-block
_JCH = _WCH // 2 # output columns per tile

_NB_EO = 5       # input-tile buffers (load lookahead)
_NB_H = 2        # stage-1 H buffers
_NB_OT = 3       # output-tile buffers

_f32 = mybir.dt.float32
_ADD = mybir.AluOpType.add
_SUB = mybir.AluOpType.subtract


def _build_nc(k: float, repeat: int = 1) -> bass.Bass:
    nc = bass.Bass()
    x = nc.dram_tensor("x", [_N, _N, _C], _f32, kind="ExternalInput")
    out = nc.dram_tensor("out", [_HALF, _HALF, 4 * _C], _f32, kind="ExternalOutput")

    # view x rows as (i, even/odd) pairs
    xr = x[:].rearrange("(i e) w c -> i e w c", e=2)  # [256, 2, 512, 32]

    units = [
        (ib, wb) for _ in range(repeat) for ib in range(_IB) for wb in range(_WB)
    ]
    n_units = len(units)

    with ExitStack() as ctx:
        eo = [
            ctx.enter_context(nc.sbuf_tensor(f"eo{i}", [128, 2, _WCH, _C], _f32))
            for i in range(_NB_EO)
        ]
        # H: stage-1 output, hs and hd interleaved on the `s` axis
        H = [
            ctx.enter_context(nc.sbuf_tensor(f"H{i}", [128, _WCH, 2, _C], _f32))
            for i in range(_NB_H)
        ]
        ot = [
            ctx.enter_context(nc.sbuf_tensor(f"ot{i}", [128, _JCH, 4, _C], _f32))
            for i in range(_NB_OT)
        ]
        s_load = [
            ctx.enter_context(nc.semaphore(f"s_load{i}")) for i in range(_NB_EO)
        ]
        s_store = [
            ctx.enter_context(nc.semaphore(f"s_store{i}")) for i in range(_NB_OT)
        ]
        s_eofree = ctx.enter_context(nc.semaphore("s_eofree"))  # hs+hd read eo
        s_hd = ctx.enter_context(nc.semaphore("s_hd"))          # Pool wrote hd
        s_st2 = ctx.enter_context(nc.semaphore("s_st2"))        # DVE stage-2 done
        s_mul = ctx.enter_context(nc.semaphore("s_mul"))        # ACT scale done
        s_bar = ctx.enter_context(nc.semaphore("s_bar"))        # end barrier
        block = ctx.enter_context(nc.Block())

        sems = s_load + s_store + [s_eofree, s_hd, s_st2, s_mul, s_bar]
        n_store_lane = [len(range(lane, n_units, _NB_OT)) for lane in range(_NB_OT)]

        @block.sync
        def _(sync):
            ch = _Chain()
            for u, (ib, wb) in enumerate(units):
                src = xr[
                    ib * 128 : (ib + 1) * 128, :, wb * _WCH : (wb + 1) * _WCH, :
                ]
                i = ch(sync.dma_start(out=eo[u % _NB_EO][:], in_=src))
                if u >= _NB_EO:
                    # previous tenant u-_NB_EO fully read: both its hs (DVE)
                    # and hd (Pool) incremented s_eofree
                    i.wait_op(s_eofree, 2 * (u - _NB_EO + 1), "sem-ge")
                i.then_inc(s_load[u % _NB_EO], 16)
            ch(sync.sem_inc(s_bar, 1))

        @block.vector
        def _(vector):
            ch = _Chain()
            for u, (ib, wb) in enumerate(units):
                b = eo[u % _NB_EO]
                h = H[u % _NB_H]
                o = ot[u % _NB_OT]
                ev = b[:, 0]  # [128, WCH, C]
                od = b[:, 1]
                # stage 1 (DVE half): hs = ev + od -> H[:, :, 0, :]
                i = ch(vector.tensor_tensor(out=h[:, :, 0], in0=ev, in1=od, op=_ADD))
                i.wait_op(s_load[u % _NB_EO], 16 * (u // _NB_EO + 1), "sem-ge")
                i.then_inc(s_eofree, 1)
                # O-buffer availability (store of tenant u-_NB_OT landed)
                if u >= _NB_OT:
                    ch(vector.wait_ge(s_store[u % _NB_OT], 16 * (u // _NB_OT)))
                # stage 2: needs Pool's hd as well
                hv = h[:].rearrange("p (j e) s c -> p j e s c", e=2)
                i = ch(
                    vector.tensor_tensor(
                        out=o[:, :, 0:2], in0=hv[:, :, 0], in1=hv[:, :, 1], op=_ADD
                    )
                )
                i.wait_op(s_hd, u + 1, "sem-ge")
                i = ch(
                    vector.tensor_tensor(
                        out=o[:, :, 2:4], in0=hv[:, :, 1], in1=hv[:, :, 0], op=_SUB
                    )
                )
                i.then_inc(s_st2, 1)
            ch(vector.sem_inc(s_bar, 1))

        @block.gpsimd
        def _(gpsimd):
            ch = _Chain()
            for u, (ib, wb) in enumerate(units):
                b = eo[u % _NB_EO]
                h = H[u % _NB_H]
                ev = b[:, 0]
                od = b[:, 1]
                # H-buffer availability: stage-2 of tenant u-_NB_H done
                if u >= _NB_H:
                    ch(gpsimd.wait_ge(s_st2, u - _NB_H + 1))
                # stage 1 (Pool half): hd = od - ev -> H[:, :, 1, :]
                i = ch(gpsimd.tensor_tensor(out=h[:, :, 1], in0=od, in1=ev, op=_SUB))
                i.wait_op(s_load[u % _NB_EO], 16 * (u // _NB_EO + 1), "sem-ge")
                i.then_inc(s_hd, 1)
                ch(gpsimd.sem_inc(s_eofree, 1))
            # end-of-stream: observe all counters, then reset for re-execution
            ch(gpsimd.wait_ge(s_bar, 3))
            for lane in range(_NB_OT):
                ch(gpsimd.wait_ge(s_store[lane], 16 * n_store_lane[lane]))
            for lane in range(_NB_EO):
                ch(gpsimd.wait_ge(s_load[lane], 16 * len(range(lane, n_units, _NB_EO))))
            ch(gpsimd.wait_ge(s_eofree, 2 * n_units))
            ch(gpsimd.wait_ge(s_st2, n_units))
            ch(gpsimd.wait_ge(s_mul, n_units))
            nums = sorted(s.num for s in sems)
            lo = nums[0]
            hi = nums[-1] + 1
            assert nums == list(range(lo, hi)), nums
            ch(gpsimd.dma_reset(range(lo, hi)))
            ch(gpsimd.sem_clear(range(lo, hi)))

        @block.scalar
        def _(scalar):
            ch = _Chain()
            for u, (ib, wb) in enumerate(units):
                o = ot[u % _NB_OT]
                otf = o[:].rearrange("p j q c -> p (j q c)")
                i = ch(scalar.mul(otf, otf, k))
                i.wait_op(s_st2, u + 1, "sem-ge")
                i.then_inc(s_mul, 1)
                dst = out[
                    ib * 128 : (ib + 1) * 128, wb * _JCH : (wb + 1) * _JCH, :
                ]
                # the HWDGE trigger would otherwise race the ACT datapath
                ch(scalar.dma_start(
                    out=dst, in_=o[:].rearrange("p j q c -> p j (q c)")
                )).wait_op(s_mul, u + 1, "sem-ge").then_inc(s_store[u % _NB_OT], 16)
            ch(scalar.sem_inc(s_bar, 1))

    return nc


def _expected_A(s: np.float32) -> np.ndarray:
    A = np.zeros((_N, _N), np.float32)
    i = np.arange(_HALF)
    A[i, 2 * i] = s
    A[i, 2 * i + 1] = s
    A[_HALF + i, 2 * i] = -s
    A[_HALF + i, 2 * i + 1] = s
    return A


def _fallback(x: np.ndarray, A: np.ndarray) -> np.ndarray:
    xt = np.transpose(x, (0, 2, 1, 3))
    y = np.einsum("ij,bjkc->bikc", A, xt, optimize=True).astype(np.float32)
    y = np.transpose(y, (0, 2, 1, 3))
    y = np.einsum("ij,bjkc->bikc", A, y, optimize=True).astype(np.float32)
    mid = y.shape[1] // 2
    return np.concatenate(
        [y[:, :mid, :mid], y[:, mid:, :mid], y[:, :mid, mid:], y[:, mid:, mid:]],
        axis=-1,
    )


def run_on_device(x: np.ndarray, k: float, trace: bool = False):
    nc = _build_nc(k)
    in_maps = [{"x": np.ascontiguousarray(x[b])} for b in range(_B)]
    res = bass_utils.run_bass_kernel_spmd(
        nc, in_maps, core_ids=list(range(_B)), trace=trace
    )
    out = np.stack([r["out"] for r in res.results], axis=0)
    return out, res


def kernel(x: np.ndarray, A: np.ndarray) -> np.ndarray:
    x = np.asarray(x, dtype=np.float32)
    A = np.asarray(A, dtype=np.float32)
    s = A[0, 0]
    if not np.array_equal(A, _expected_A(s)):
        return _fallback(x, A)
    k = float(np.float32(s) * np.float32(s))
    out, _ = run_on_device(x, k)
    return out


# revision 3
# speedup vs baseline: 1.0652x; 1.0336x over previous
"""2D Haar DWT (analysis) kernel for Trainium2, 8 NeuronCores — V5.

V3 compute structure (DVE hs + Pool hd stage-1 into one interleaved H tile,
2-op merged stage-2, ACT scale + store), but input loads re-tiled to 4MiB
(w-chunk 128 -> 16KB descriptors, 8 dma_starts instead of 16) to cut input
descriptor count toward the pure-DMA streaming floor. Each 4MiB load feeds
two 2MiB compute sub-units.
"""

from contextlib import ExitStack

import numpy as np

import concourse.bass as bass
import concourse.mybir as mybir
from concourse import bass_utils
from concourse.instruction_name_ordered_set import InstructionNameOrderedSet


class _Chain:
    def __init__(self):
        self.prev = None

    def __call__(self, inst):
        if self.prev is not None:
            inst.ins.set_nosync_dependencies(
                InstructionNameOrderedSet([self.prev])
            )
        self.prev = inst.ins.name
        return inst


_B = 8
_N = 512
_C = 32
_HALF = _N // 2

_IB = 2            # i-blocks of 128 output rows each
_LCH = 128         # input-width columns per LOAD tile (4MiB)
_LB = _N // _LCH   # loads per i-block (4)
_WCH = 64          # input-width columns per compute sub-unit
_JCH = _WCH // 2   # output columns per sub-unit

_NB_EO = 3         # 4MiB input buffers
_NB_H = 2
_NB_OT = 3

_f32 = mybir.dt.float32
_ADD = mybir.AluOpType.add
_SUB = mybir.AluOpType.subtract


def _build_nc(k: float, repeat: int = 1) -> bass.Bass:
    nc = bass.Bass()
    x = nc.dram_tensor("x", [_N, _N, _C], _f32, kind="ExternalInput")
    out = nc.dram_tensor("out", [_HALF, _HALF, 4 * _C], _f32, kind="ExternalOutput")

    xr = x[:].rearrange("(i e) w c -> i e w c", e=2)  # [256, 2, 512, 32]

    # compute sub-units: (ib, wb) with wb in 0..7 over 64-wide w-chunks;
    # load l = u // 2 covers w-chunks (2q, 2q+1)
    units = [
        (ib, wb)
        for _ in range(repeat)
        for ib in range(_IB)
        for wb in range(_N // _WCH)
    ]
    loads = [
        (ib, q) for _ in range(repeat) for ib in range(_IB) for q in range(_LB)
    ]
    n_units = len(units)
    n_loads = len(loads)

    with ExitStack() as ctx:
        eo = [
            ctx.enter_context(nc.sbuf_tensor(f"eo{i}", [128, 2, _LCH, _C], _f32))
            for i in range(_NB_EO)
        ]
        H = [
            ctx.enter_context(nc.sbuf_tensor(f"H{i}", [128, _WCH, 2, _C], _f32))
            for i in range(_NB_H)
        ]
        ot = [
            ctx.enter_context(nc.sbuf_tensor(f"ot{i}", [128, _JCH, 4, _C], _f32))
            for i in range(_NB_OT)
        ]
        s_load = [
            ctx.enter_context(nc.semaphore(f"s_load{i}")) for i in range(_NB_EO)
        ]
        s_store = [
            ctx.enter_context(nc.semaphore(f"s_store{i}")) for i in range(_NB_OT)
        ]
        s_hd = ctx.enter_context(nc.semaphore("s_hd"))
        s_st2 = ctx.enter_context(nc.semaphore("s_st2"))
        s_mul = ctx.enter_context(nc.semaphore("s_mul"))
        s_bar = ctx.enter_context(nc.semaphore("s_bar"))
        block = ctx.enter_context(nc.Block())

        sems = s_load + s_store + [s_hd, s_st2, s_mul, s_bar]
        n_store_lane = [len(range(lane, n_units, _NB_OT)) for lane in range(_NB_OT)]
        n_load_lane = [len(range(lane, n_loads, _NB_EO)) for lane in range(_NB_EO)]

        def eo_slice(u):
            """Sub-unit u's [128, 2, 64, 32] view of its 4MiB load buffer."""
            l = u // 2
            half = u % 2
            return eo[l % _NB_EO][:, :, half * _WCH : (half + 1) * _WCH, :]

        @block.sync
        def _(sync):
            ch = _Chain()
            for l, (ib, q) in enumerate(loads):
                src = xr[
                    ib * 128 : (ib + 1) * 128, :, q * _LCH : (q + 1) * _LCH, :
                ]
                i = ch(sync.dma_start(out=eo[l % _NB_EO][:], in_=src))
                if l >= _NB_EO:
                    # previous tenant l-_NB_EO fully read: stage-2 completion
                    # of its last sub-unit implies both hs (program order)
                    # and hd (s_hd wait) consumed the buffer
                    i.wait_op(s_st2, 2 * (l - _NB_EO) + 2, "sem-ge")
                i.then_inc(s_load[l % _NB_EO], 16)
            ch(sync.sem_inc(s_bar, 1))

        @block.vector
        def _(vector):
            ch = _Chain()
            for u, (ib, wb) in enumerate(units):
                l = u // 2
                b = eo_slice(u)
                h = H[u % _NB_H]
                o = ot[u % _NB_OT]
                ev = b[:, 0]  # [128, WCH, C]
                od = b[:, 1]
                i = ch(vector.tensor_tensor(out=h[:, :, 0], in0=ev, in1=od, op=_ADD))
                i.wait_op(s_load[l % _NB_EO], 16 * (l // _NB_EO + 1), "sem-ge")
                if u >= _NB_OT:
                    ch(vector.wait_ge(s_store[u % _NB_OT], 16 * (u // _NB_OT)))
                hv = h[:].rearrange("p (j e) s c -> p j e s c", e=2)
                i = ch(
                    vector.tensor_tensor(
                        out=o[:, :, 0:2], in0=hv[:, :, 0], in1=hv[:, :, 1], op=_ADD
                    )
                )
                i.wait_op(s_hd, u + 1, "sem-ge")
                i = ch(
                    vector.tensor_tensor(
                        out=o[:, :, 2:4], in0=hv[:, :, 1], in1=hv[:, :, 0], op=_SUB
                    )
                )
                i.then_inc(s_st2, 1)
            ch(vector.sem_inc(s_bar, 1))

        @block.gpsimd
        def _(gpsimd):
            ch = _Chain()
            for u, (ib, wb) in enumerate(units):
                l = u // 2
                b = eo_slice(u)
                h = H[u % _NB_H]
                ev = b[:, 0]
                od = b[:, 1]
                if u >= _NB_H:
                    ch(gpsimd.wait_ge(s_st2, u - _NB_H + 1))
                i = ch(gpsimd.tensor_tensor(out=h[:, :, 1], in0=od, in1=ev, op=_SUB))
                i.wait_op(s_load[l % _NB_EO], 16 * (l // _NB_EO + 1), "sem-ge")
                i.then_inc(s_hd, 1)
            ch(gpsimd.wait_ge(s_bar, 3))
            for lane in range(_NB_OT):
                ch(gpsimd.wait_ge(s_store[lane], 16 * n_store_lane[lane]))
            for lane in range(_NB_EO):
                ch(gpsimd.wait_ge(s_load[lane], 16 * n_load_lane[lane]))
            ch(gpsimd.wait_ge(s_st2, n_units))
            ch(gpsimd.wait_ge(s_mul, n_units))
            nums = sorted(s.num for s in sems)
            lo = nums[0]
            hi = nums[-1] + 1
            assert nums == list(range(lo, hi)), nums
            ch(gpsimd.dma_reset(range(lo, hi)))
            ch(gpsimd.sem_clear(range(lo, hi)))

        @block.scalar
        def _(scalar):
            ch = _Chain()
            for u, (ib, wb) in enumerate(units):
                o = ot[u % _NB_OT]
                otf = o[:].rearrange("p j q c -> p (j q c)")
                i = ch(scalar.mul(otf, otf, k))
                i.wait_op(s_st2, u + 1, "sem-ge")
                i.then_inc(s_mul, 1)
                dst = out[
                    ib * 128 : (ib + 1) * 128, wb * _JCH : (wb + 1) * _JCH, :
                ]
                ch(scalar.dma_start(
                    out=dst, in_=o[:].rearrange("p j q c -> p j (q c)")
                )).wait_op(s_mul, u + 1, "sem-ge").then_inc(s_store[u % _NB_OT], 16)
            ch(scalar.sem_inc(s_bar, 1))

    return nc


def _expected_A(s: np.float32) -> np.ndarray:
    A = np.zeros((_N, _N), np.float32)
    i = np.arange(_HALF)
    A[i, 2 * i] = s
    A[i, 2 * i + 1] = s
    A[_HALF + i, 2 * i] = -s
    A[_HALF + i, 2 * i + 1] = s
    return A


def _fallback(x: np.ndarray, A: np.ndarray) -> np.ndarray:
    xt = np.transpose(x, (0, 2, 1, 3))
    y = np.einsum("ij,bjkc->bikc", A, xt, optimize=True).astype(np.float32)
    y = np.transpose(y, (0, 2, 1, 3))
    y = np.einsum("ij,bjkc->bikc", A, y, optimize=True).astype(np.float32)
    mid = y.shape[1] // 2
    return np.concatenate(
        [y[:, :mid, :mid], y[:, mid:, :mid], y[:, :mid, mid:], y[:, mid:, mid:]],
        axis=-1,
    )


def run_on_device(x: np.ndarray, k: float, trace: bool = False):
    nc = _build_nc(k)
    in_maps = [{"x": np.ascontiguousarray(x[b])} for b in range(_B)]
    res = bass_utils.run_bass_kernel_spmd(
        nc, in_maps, core_ids=list(range(_B)), trace=trace
    )
    out = np.stack([r["out"] for r in res.results], axis=0)
    return out, res


def kernel(x: np.ndarray, A: np.ndarray) -> np.ndarray:
    x = np.asarray(x, dtype=np.float32)
    A = np.asarray(A, dtype=np.float32)
    s = A[0, 0]
    if not np.array_equal(A, _expected_A(s)):
        return _fallback(x, A)
    k = float(np.float32(s) * np.float32(s))
    out, _ = run_on_device(x, k)
    return out
